# revision 20
# baseline (speedup 1.0000x reference)
"""MoE LoRA adapter layer (top-2 routed, E=8 experts, R=16) on 8 TRN2 NeuronCores.

Strategy: data-parallel over batch B=32 -> 4 batches/core; router + LoRA
weights replicated (tiny). E*R = 128 = partition width, so the per-expert
LoRA down/up projections stack into two dense matmuls:
    P1[er, t] = D_all[er, :] @ x[t, :]^T          (contract H=1024)
    wT[h, t]  = sum_er U_all[er, h] * (gate[b(t), e(er)] * P1[er, t])
The expert sum IS the matmul contraction; gates (exactly 0 off the top-2)
are folded in by scaling P1 columns. out = x + w.

Layout: x is shipped ALREADY TRANSPOSED (h-major) from the host, so the
kernel needs zero PE transposes: MM1 consumes xT directly and MM2 produces
outT in the same h-major layout the store expects. Per chunk (= one batch,
512 tokens) the input slice is one 8 KiB contiguous run per partition.
PSUM->SBUF eviction of the result (the residual add) is split between the
vector and scalar engines so neither becomes the bottleneck. Gates are
computed on-device in fp32 (exact top-2) from a tiny pre-transposed cls.
"""

import sys

if "/opt/trn_rl_repo" not in sys.path:
    sys.path.insert(0, "/opt/trn_rl_repo")

import numpy as np
import ml_dtypes

import concourse.bass as bass
import concourse.tile as tile
from concourse import bacc, mybir
from concourse.bass_utils import run_bass_kernel_spmd

B, L, H = 32, 512, 1024
E, R, TOP_K = 8, 16, 2
N_CORES = 8
NB = B // N_CORES          # batches per core = 4
T = NB * L                 # tokens per core = 2048
P = 128                    # partitions
NK = H // P                # H k-tiles = 8
C = NB                     # chunks per core (one batch = 512 tokens each)
CT = L                     # tokens per chunk

# eviction k-slices that go PSUM->vector directly; the rest take the
# scalar-copy + bf16-add path so scalar shares the eviction load
VEC_DIRECT = (0, 2, 4, 6)

F32 = mybir.dt.float32
BF16 = mybir.dt.bfloat16
BF16_NP = ml_dtypes.bfloat16

_COMPILED = None


def _build():
    """Build + compile the single-core program (same on all 8 cores)."""
    nc = bacc.Bacc("TRN2", target_bir_lowering=False, debug=False)

    # weights packed into 3 tensors (DMA issues cost ~650ns each on the
    # issuing engine, so fewer+parallel-queue issues shorten the head)
    x_in = nc.dram_tensor("x_in", [P, C * NK * CT], BF16, kind="ExternalInput")
    wf32 = nc.dram_tensor("wf32", [P, NK * NB + NK * E], F32, kind="ExternalInput")
    wbf = nc.dram_tensor("wbf", [P, 2 * H], BF16, kind="ExternalInput")
    w8 = nc.dram_tensor("w8", [8, 8 + P], F32, kind="ExternalInput")
    y_out = nc.dram_tensor("y_out", [P, C * NK * CT], BF16, kind="ExternalOutput")

    # (p, c, k, t): chunk c, h-tile k, token t -> xT[k*128+p, c*512+t]
    # halves split the chunk by k (0..3 | 4..7): each is 4 KiB/partition
    x_hap = x_in.ap().rearrange("p (c h f) -> c h p f", c=C, h=2)
    # stores go out per (chunk, k-pair): 2 KiB contiguous per partition
    y_ap = y_out.ap().rearrange("p (c g f) -> c g p f", c=C, g=NK // 2)

    with tile.TileContext(nc) as tc:
        with (
            tc.tile_pool(name="wpool", bufs=1) as wpool,
            tc.tile_pool(name="gpool", bufs=1) as gpool,
            tc.tile_pool(name="xpool", bufs=C) as xpool,
            tc.tile_pool(name="opool", bufs=3) as opool,
            tc.tile_pool(name="p2pool", bufs=2) as p2pool,
            tc.tile_pool(name="wbpool", bufs=4) as wbpool,
            tc.tile_pool(name="p1_ps", bufs=2, space="PSUM") as p1_ps,
            tc.tile_pool(name="w_ps", bufs=6, space="PSUM") as w_ps,
        ):
            # ---- loads on three parallel queues so no single engine's
            # ~650ns-per-issue serial cost delays the stream, and weights
            # never queue behind the big x transfers:
            #   scalar (HWDGE): wf32 (cls+rwt), wbf (d+u)
            #   sync   (HWDGE): x chunk0 half0, w8 (id8+rep), later stores
            #   gpsimd (SWDGE): remaining 7 x halves
            wf_sb = wpool.tile([P, NK * NB + NK * E], F32, tag="wf")
            nc.scalar.dma_start(wf_sb[:], wf32.ap())
            wb_sb = wpool.tile([P, 2 * H], BF16, tag="wb")
            nc.scalar.dma_start(wb_sb[:], wbf.ap())
            RW0 = NK * NB          # rwt column base in wf_sb

            x_tiles = []
            for _c in range(C):
                xb = xpool.tile([P, NK * CT], BF16, tag="xb")
                x_tiles.append(xb)

            def load_half(eng, c, h):
                eng.dma_start(
                    x_tiles[c][:, h * (NK // 2) * CT : (h + 1) * (NK // 2) * CT],
                    x_hap[c, h],
                )

            load_half(nc.sync, 0, 0)
            w8_sb = wpool.tile([8, 8 + P], F32, tag="w8")
            nc.sync.dma_start(w8_sb[:], w8.ap())
            load_half(nc.gpsimd, 0, 1)
            for c in range(1, C):
                load_half(nc.gpsimd, c, 0)
                load_half(nc.gpsimd, c, 1)

            holders = {}

            def stage_prologue():
                # logits [NB, E] = cls @ router_w^T, contracted over H
                lg_ps = p1_ps.tile([P, 512], F32, tag="p1")
                for k in range(NK):
                    nc.tensor.matmul(
                        lg_ps[0:NB, 0:E],
                        wf_sb[:, k * NB : (k + 1) * NB],
                        wf_sb[:, RW0 + k * E : RW0 + (k + 1) * E],
                        start=(k == 0),
                        stop=(k == NK - 1),
                    )
                lg = gpool.tile([NB, E], F32, tag="lg")
                nc.vector.tensor_copy(lg[:], lg_ps[0:NB, 0:E])

                # top-2 softmax per row (E=8 along free dim)
                m1 = gpool.tile([NB, 1], F32, tag="m1")
                nc.vector.reduce_max(m1[:], lg[:], axis=mybir.AxisListType.X)
                t_sb = gpool.tile([NB, E], F32, tag="t")
                nc.vector.tensor_scalar(
                    t_sb[:], lg[:], m1[:], None, op0=mybir.AluOpType.subtract
                )
                # pen = (t >= 0) * 1e30  (knocks out the argmax)
                pen = gpool.tile([NB, E], F32, tag="pen")
                nc.vector.tensor_scalar(
                    pen[:], t_sb[:], 0.0, 1e30,
                    op0=mybir.AluOpType.is_ge, op1=mybir.AluOpType.mult,
                )
                t2 = gpool.tile([NB, E], F32, tag="t2")
                nc.vector.tensor_sub(t2[:], t_sb[:], pen[:])
                m2 = gpool.tile([NB, 1], F32, tag="m2")
                nc.vector.reduce_max(m2[:], t2[:], axis=mybir.AxisListType.X)
                keep = gpool.tile([NB, E], F32, tag="keep")
                nc.vector.tensor_scalar(
                    keep[:], t_sb[:], m2[:], None, op0=mybir.AluOpType.is_ge
                )
                ex = gpool.tile([NB, E], F32, tag="ex")
                nc.scalar.activation(ex[:], t_sb[:], mybir.ActivationFunctionType.Exp)
                eg = gpool.tile([NB, E], F32, tag="eg")
                nc.vector.tensor_mul(eg[:], ex[:], keep[:])
                s_sb = gpool.tile([NB, 1], F32, tag="s")
                nc.vector.reduce_sum(s_sb[:], eg[:], axis=mybir.AxisListType.X)
                rs = gpool.tile([NB, 1], F32, tag="rs")
                nc.vector.reciprocal(rs[:], s_sb[:])
                gts = gpool.tile([NB, E], F32, tag="gts")
                nc.vector.tensor_scalar(
                    gts[:], eg[:], rs[:], None, op0=mybir.AluOpType.mult
                )

                # gatesT then replicate x16 along partitions -> gvec [128, NB]
                gt_ps = p1_ps.tile([P, 512], F32, tag="p1")
                nc.tensor.transpose(gt_ps[0:E, 0:NB], gts[:], w8_sb[0:NB, 0:NB])
                gtT = gpool.tile([E, NB], F32, tag="gtT")
                nc.vector.tensor_copy(gtT[:], gt_ps[0:E, 0:NB])
                gv_ps = p1_ps.tile([P, 512], F32, tag="p1")
                nc.tensor.matmul(gv_ps[:, 0:NB], w8_sb[0:E, 8 : 8 + P], gtT[:])
                gvec = gpool.tile([P, NB], F32, tag="gvec")
                nc.vector.tensor_copy(gvec[:], gv_ps[:, 0:NB])
                holders["gvec"] = gvec

            p2_tiles = {}

            def stage_mm1(c):
                p1 = p1_ps.tile([P, CT], F32, tag="p1")
                for k in range(NK):
                    nc.tensor.matmul(
                        p1[:],
                        wb_sb[:, k * P : (k + 1) * P],
                        x_tiles[c][:, k * CT : (k + 1) * CT],
                        start=(k == 0),
                        stop=(k == NK - 1),
                    )
                p2 = p2pool.tile([P, CT], BF16, tag="p2")
                nc.scalar.activation(
                    p2[:], p1[:], mybir.ActivationFunctionType.Copy,
                    scale=holders["gvec"][:, c : c + 1],
                )
                p2_tiles[c] = p2

            # eviction engine per (late?, k): v=vector direct add from PSUM,
            # s=scalar copy + bf16 add on the engine given by ADD_ENG.
            # GPSIMD cannot touch PSUM on TRN2 (BIR verifier enforces it),
            # so it only takes bf16 SBUF adds, and only for chunks 2-3 --
            # early on it is busy issuing the x-load DMAs.
            EVICT = {
                False: "vsvsvsvs",   # chunks 0-1
                True: "vsvsvsvs",    # chunks 2-3
            }
            ADD_ENG = {
                False: {1: "v", 3: "v", 5: "v", 7: "v"},
                True: {1: "g", 3: "g", 5: "g", 7: "g"},
            }

            def stage_mm2(c):
                late = c >= 2
                o_sb = opool.tile([P, NK * CT], BF16, tag="o")
                for k in range(NK):
                    wps = w_ps.tile([P, CT], F32, tag="w")
                    nc.tensor.matmul(
                        wps[:],
                        wb_sb[:, H + k * P : H + (k + 1) * P],
                        p2_tiles[c][:],
                    )
                    x_k = x_tiles[c][:, k * CT : (k + 1) * CT]
                    o_k = o_sb[:, k * CT : (k + 1) * CT]
                    mode = EVICT[late][k]
                    if mode == "v":
                        nc.vector.tensor_add(o_k, wps[:], x_k)
                    else:
                        wb = wbpool.tile([P, CT], BF16, tag="wb")
                        nc.scalar.activation(
                            wb[:], wps[:], mybir.ActivationFunctionType.Copy
                        )
                        if ADD_ENG[late][k] == "g":
                            nc.gpsimd.tensor_add(o_k, wb[:], x_k)
                        else:
                            nc.vector.tensor_add(o_k, wb[:], x_k)
                    if k % 2 == 1:
                        nc.sync.dma_start(
                            y_ap[c, k // 2], o_sb[:, (k - 1) * CT : (k + 1) * CT]
                        )

            stage_prologue()
            stage_mm1(0)
            for c in range(1, C):
                stage_mm1(c)
                stage_mm2(c - 1)
            stage_mm2(C - 1)

    nc.compile()
    return nc


def _weights_maps(router_w, lora_down, lora_up):
    # D_all[(e,r), h] stacked; lhsT tiles need [p, k, m] = D_all[m, k*128+p]
    d_all = lora_down.reshape(E * R, H)                       # [128, 1024]
    d_t = np.ascontiguousarray(
        d_all.T.reshape(NK, P, E * R).transpose(1, 0, 2).reshape(P, NK * P)
    ).astype(BF16_NP)
    # U_all[(e,r), h] = lora_up[e, h, r]
    u_np = np.ascontiguousarray(
        lora_up.transpose(0, 2, 1).reshape(E * R, H)
    ).astype(BF16_NP)
    # router_wT tiles [p, k, e] = router_w[e, k*128+p]
    rwt_np = np.ascontiguousarray(
        router_w.T.reshape(NK, P, E).transpose(1, 0, 2).reshape(P, NK * E)
    ).astype(np.float32)
    rep_np = np.zeros((E, P), np.float32)
    for e in range(E):
        rep_np[e, e * R : (e + 1) * R] = 1.0
    w8_np = np.concatenate([np.eye(8, dtype=np.float32), rep_np], axis=1)
    wbf_np = np.ascontiguousarray(np.concatenate([d_t, u_np], axis=1))
    return rwt_np, wbf_np, w8_np


def get_compiled():
    global _COMPILED
    if _COMPILED is None:
        _COMPILED = _build()
    return _COMPILED


def make_in_maps(x, router_w, lora_down, lora_up):
    x = np.asarray(x, np.float32)
    rwt_np, wbf_np, w8_np = _weights_maps(
        np.asarray(router_w, np.float32),
        np.asarray(lora_down, np.float32),
        np.asarray(lora_up, np.float32),
    )
    in_maps = []
    for i in range(N_CORES):
        xs = x[i * NB : (i + 1) * NB]                         # [C, CT, H]
        # (p, c, k, t) <- xs[c, t, k*128+p]
        xtd = np.ascontiguousarray(
            xs.reshape(C, CT, NK, P).transpose(3, 0, 2, 1).reshape(P, C * NK * CT)
        ).astype(BF16_NP)
        cls = xs[:, 0, :]                                     # [NB, H]
        cls_t = np.ascontiguousarray(
            cls.reshape(NB, NK, P).transpose(2, 1, 0).reshape(P, NK * NB)
        ).astype(np.float32)
        wf32_np = np.ascontiguousarray(np.concatenate([cls_t, rwt_np], axis=1))
        in_maps.append(
            {"x_in": xtd, "wf32": wf32_np, "wbf": wbf_np, "w8": w8_np}
        )
    return in_maps


def unshard_one(y_np):
    """[P, C*NK*CT] h-major device output -> [NB, L, H] float32."""
    y = np.asarray(y_np, np.float32).reshape(P, C, NK, CT)
    return np.ascontiguousarray(y.transpose(1, 3, 2, 0)).reshape(NB, L, H)


def kernel(x, router_w, lora_down, lora_up):
    nc = get_compiled()
    in_maps = make_in_maps(x, router_w, lora_down, lora_up)
    res = run_bass_kernel_spmd(nc, in_maps, core_ids=list(range(N_CORES)))
    out = np.empty((B, L, H), np.float32)
    for i in range(N_CORES):
        out[i * NB : (i + 1) * NB] = unshard_one(res.results[i]["y_out"])
    return out


# revision 22
# speedup vs baseline: 1.1803x; 1.1803x over previous
"""MoE LoRA adapter layer (top-2 routed, E=8 experts, R=16) on 8 TRN2 NeuronCores.

Strategy: data-parallel over batch B=32 -> 4 batches/core; router + LoRA
weights replicated (tiny). E*R = 128 = partition width, so the per-expert
LoRA down/up projections stack into two dense matmuls:
    P1[er, t] = D_all[er, :] @ x[t, :]^T          (contract H=1024)
    wT[h, t]  = sum_er U_all[er, h] * (gate[b(t), e(er)] * P1[er, t])
The expert sum IS the matmul contraction; gates (exactly 0 off the top-2)
are folded in by scaling P1 columns. out = x + w.

Layout: x is shipped ALREADY TRANSPOSED (h-major) from the host, so the
kernel needs zero PE transposes: MM1 consumes xT directly and MM2 produces
outT in the same h-major layout the store expects. Per chunk (= one batch,
512 tokens) the input slice is one 8 KiB contiguous run per partition.
PSUM->SBUF eviction of the result (the residual add) is split between the
vector and scalar engines so neither becomes the bottleneck. Gates are
computed on-device in fp32 (exact top-2) from a tiny pre-transposed cls.
"""

import sys

if "/opt/trn_rl_repo" not in sys.path:
    sys.path.insert(0, "/opt/trn_rl_repo")

import numpy as np
import ml_dtypes

import concourse.bass as bass
import concourse.tile as tile
from concourse import bacc, mybir
from concourse.bass_utils import run_bass_kernel_spmd

B, L, H = 32, 512, 1024
E, R, TOP_K = 8, 16, 2
N_CORES = 8
NB = B // N_CORES          # batches per core = 4
T = NB * L                 # tokens per core = 2048
P = 128                    # partitions
NK = H // P                # H k-tiles = 8
C = NB                     # chunks per core (one batch = 512 tokens each)
CT = L                     # tokens per chunk

# eviction k-slices that go PSUM->vector directly; the rest take the
# scalar-copy + bf16-add path so scalar shares the eviction load
VEC_DIRECT = (0, 2, 4, 6)

F32 = mybir.dt.float32
BF16 = mybir.dt.bfloat16
BF16_NP = ml_dtypes.bfloat16

_COMPILED = None


def _build():
    """Build + compile the single-core program (same on all 8 cores)."""
    nc = bacc.Bacc("TRN2", target_bir_lowering=False, debug=False)

    # weights packed into 3 tensors (DMA issues cost ~650ns each on the
    # issuing engine, so fewer+parallel-queue issues shorten the head)
    x_in = nc.dram_tensor("x_in", [P, C * NK * CT], BF16, kind="ExternalInput")
    wf32 = nc.dram_tensor("wf32", [P, NK * NB + NK * E], F32, kind="ExternalInput")
    wbf = nc.dram_tensor("wbf", [P, 2 * H], BF16, kind="ExternalInput")
    w8 = nc.dram_tensor("w8", [8, 8 + P], F32, kind="ExternalInput")
    y_out = nc.dram_tensor("y_out", [P, C * NK * CT], BF16, kind="ExternalOutput")

    # (p, c, k, t): chunk c, h-tile k, token t -> xT[k*128+p, c*512+t]
    # halves split the chunk by k (0..3 | 4..7): each is 4 KiB/partition
    x_hap = x_in.ap().rearrange("p (c h f) -> c h p f", c=C, h=2)
    # stores go out per (chunk, k-pair): 2 KiB contiguous per partition
    y_ap = y_out.ap().rearrange("p (c g f) -> c g p f", c=C, g=NK // 2)

    with tile.TileContext(nc) as tc:
        with (
            tc.tile_pool(name="wpool", bufs=1) as wpool,
            tc.tile_pool(name="gpool", bufs=1) as gpool,
            tc.tile_pool(name="xpool", bufs=C) as xpool,
            tc.tile_pool(name="opool", bufs=3) as opool,
            tc.tile_pool(name="p2pool", bufs=2) as p2pool,
            tc.tile_pool(name="wbpool", bufs=4) as wbpool,
            tc.tile_pool(name="p1_ps", bufs=2, space="PSUM") as p1_ps,
            tc.tile_pool(name="w_ps", bufs=6, space="PSUM") as w_ps,
        ):
            # ---- loads: ALL on the gpsimd (SWDGE) queue. Splitting loads
            # across queues measured WORSE (descriptors of all active
            # queues interleave round-robin, collapsing the ramp to
            # ~150GB/s), and transfer completion follows bytes-enqueued
            # order, not FIFO, so the only control is enqueue order:
            # packed weights first, then x half-chunks in pipeline order.
            wf_sb = wpool.tile([P, NK * NB + NK * E], F32, tag="wf")
            nc.gpsimd.dma_start(wf_sb[:], wf32.ap())
            wb_sb = wpool.tile([P, 2 * H], BF16, tag="wb")
            nc.gpsimd.dma_start(wb_sb[:], wbf.ap())
            w8_sb = wpool.tile([8, 8 + P], F32, tag="w8")
            nc.gpsimd.dma_start(w8_sb[:], w8.ap())
            RW0 = NK * NB          # rwt column base in wf_sb

            x_tiles = []
            for _c in range(C):
                xb = xpool.tile([P, NK * CT], BF16, tag="xb")
                x_tiles.append(xb)

            def load_half(c, h):
                nc.gpsimd.dma_start(
                    x_tiles[c][:, h * (NK // 2) * CT : (h + 1) * (NK // 2) * CT],
                    x_hap[c, h],
                )

            for c in range(C):
                load_half(c, 0)
                load_half(c, 1)

            holders = {}

            def stage_prologue():
                # logits [NB, E] = cls @ router_w^T, contracted over H
                lg_ps = p1_ps.tile([P, 512], F32, tag="p1")
                for k in range(NK):
                    nc.tensor.matmul(
                        lg_ps[0:NB, 0:E],
                        wf_sb[:, k * NB : (k + 1) * NB],
                        wf_sb[:, RW0 + k * E : RW0 + (k + 1) * E],
                        start=(k == 0),
                        stop=(k == NK - 1),
                    )
                # top-2 softmax per row (E=8 along free dim); logits read
                # straight from PSUM to skip a serial copy
                m1 = gpool.tile([NB, 1], F32, tag="m1")
                nc.vector.reduce_max(
                    m1[:], lg_ps[0:NB, 0:E], axis=mybir.AxisListType.X
                )
                t_sb = gpool.tile([NB, E], F32, tag="t")
                nc.vector.tensor_scalar(
                    t_sb[:], lg_ps[0:NB, 0:E], m1[:], None,
                    op0=mybir.AluOpType.subtract,
                )
                # pen = (t >= 0) * 1e30  (knocks out the argmax)
                pen = gpool.tile([NB, E], F32, tag="pen")
                nc.vector.tensor_scalar(
                    pen[:], t_sb[:], 0.0, 1e30,
                    op0=mybir.AluOpType.is_ge, op1=mybir.AluOpType.mult,
                )
                t2 = gpool.tile([NB, E], F32, tag="t2")
                nc.vector.tensor_sub(t2[:], t_sb[:], pen[:])
                m2 = gpool.tile([NB, 1], F32, tag="m2")
                nc.vector.reduce_max(m2[:], t2[:], axis=mybir.AxisListType.X)
                keep = gpool.tile([NB, E], F32, tag="keep")
                nc.vector.tensor_scalar(
                    keep[:], t_sb[:], m2[:], None, op0=mybir.AluOpType.is_ge
                )
                ex = gpool.tile([NB, E], F32, tag="ex")
                nc.scalar.activation(ex[:], t_sb[:], mybir.ActivationFunctionType.Exp)
                eg = gpool.tile([NB, E], F32, tag="eg")
                nc.vector.tensor_mul(eg[:], ex[:], keep[:])
                s_sb = gpool.tile([NB, 1], F32, tag="s")
                nc.vector.reduce_sum(s_sb[:], eg[:], axis=mybir.AxisListType.X)
                rs = gpool.tile([NB, 1], F32, tag="rs")
                nc.vector.reciprocal(rs[:], s_sb[:])
                gts = gpool.tile([NB, E], F32, tag="gts")
                nc.vector.tensor_scalar(
                    gts[:], eg[:], rs[:], None, op0=mybir.AluOpType.mult
                )

                # gatesT then replicate x16 along partitions -> gvec [128, NB]
                gt_ps = p1_ps.tile([P, 512], F32, tag="p1")
                nc.tensor.transpose(gt_ps[0:E, 0:NB], gts[:], w8_sb[0:NB, 0:NB])
                gtT = gpool.tile([E, NB], F32, tag="gtT")
                nc.vector.tensor_copy(gtT[:], gt_ps[0:E, 0:NB])
                gv_ps = p1_ps.tile([P, 512], F32, tag="p1")
                nc.tensor.matmul(gv_ps[:, 0:NB], w8_sb[0:E, 8 : 8 + P], gtT[:])
                gvec = gpool.tile([P, NB], F32, tag="gvec")
                nc.vector.tensor_copy(gvec[:], gv_ps[:, 0:NB])
                holders["gvec"] = gvec

            p2_tiles = {}

            def stage_mm1(c):
                p1 = p1_ps.tile([P, CT], F32, tag="p1")
                for k in range(NK):
                    nc.tensor.matmul(
                        p1[:],
                        wb_sb[:, k * P : (k + 1) * P],
                        x_tiles[c][:, k * CT : (k + 1) * CT],
                        start=(k == 0),
                        stop=(k == NK - 1),
                    )
                p2 = p2pool.tile([P, CT], BF16, tag="p2")
                nc.scalar.activation(
                    p2[:], p1[:], mybir.ActivationFunctionType.Copy,
                    scale=holders["gvec"][:, c : c + 1],
                )
                p2_tiles[c] = p2

            # eviction engine per (late?, k): v=vector direct add from PSUM,
            # s=scalar copy + bf16 add on the engine given by ADD_ENG.
            # GPSIMD cannot touch PSUM on TRN2 (BIR verifier enforces it),
            # so it only takes bf16 SBUF adds, and only for chunks 2-3 --
            # early on it is busy issuing the x-load DMAs.
            EVICT = {
                False: "vsvsvsvs",   # chunks 0-1
                True: "vsvsvsvs",    # chunks 2-3
            }
            ADD_ENG = {
                False: {1: "v", 3: "v", 5: "v", 7: "v"},
                True: {1: "g", 3: "g", 5: "g", 7: "g"},
            }

            def stage_mm2(c):
                late = c >= 2
                o_sb = opool.tile([P, NK * CT], BF16, tag="o")
                for k in range(NK):
                    wps = w_ps.tile([P, CT], F32, tag="w")
                    nc.tensor.matmul(
                        wps[:],
                        wb_sb[:, H + k * P : H + (k + 1) * P],
                        p2_tiles[c][:],
                    )
                    x_k = x_tiles[c][:, k * CT : (k + 1) * CT]
                    o_k = o_sb[:, k * CT : (k + 1) * CT]
                    mode = EVICT[late][k]
                    if mode == "v":
                        nc.vector.tensor_add(o_k, wps[:], x_k)
                    else:
                        wb = wbpool.tile([P, CT], BF16, tag="wb")
                        nc.scalar.activation(
                            wb[:], wps[:], mybir.ActivationFunctionType.Copy
                        )
                        if ADD_ENG[late][k] == "g":
                            nc.gpsimd.tensor_add(o_k, wb[:], x_k)
                        else:
                            nc.vector.tensor_add(o_k, wb[:], x_k)
                    if k % 2 == 1:
                        nc.sync.dma_start(
                            y_ap[c, k // 2], o_sb[:, (k - 1) * CT : (k + 1) * CT]
                        )

            stage_prologue()
            stage_mm1(0)
            for c in range(1, C):
                stage_mm1(c)
                stage_mm2(c - 1)
            stage_mm2(C - 1)

    nc.compile()
    return nc


def _weights_maps(router_w, lora_down, lora_up):
    # D_all[(e,r), h] stacked; lhsT tiles need [p, k, m] = D_all[m, k*128+p]
    d_all = lora_down.reshape(E * R, H)                       # [128, 1024]
    d_t = np.ascontiguousarray(
        d_all.T.reshape(NK, P, E * R).transpose(1, 0, 2).reshape(P, NK * P)
    ).astype(BF16_NP)
    # U_all[(e,r), h] = lora_up[e, h, r]
    u_np = np.ascontiguousarray(
        lora_up.transpose(0, 2, 1).reshape(E * R, H)
    ).astype(BF16_NP)
    # router_wT tiles [p, k, e] = router_w[e, k*128+p]
    rwt_np = np.ascontiguousarray(
        router_w.T.reshape(NK, P, E).transpose(1, 0, 2).reshape(P, NK * E)
    ).astype(np.float32)
    rep_np = np.zeros((E, P), np.float32)
    for e in range(E):
        rep_np[e, e * R : (e + 1) * R] = 1.0
    w8_np = np.concatenate([np.eye(8, dtype=np.float32), rep_np], axis=1)
    wbf_np = np.ascontiguousarray(np.concatenate([d_t, u_np], axis=1))
    return rwt_np, wbf_np, w8_np


def get_compiled():
    global _COMPILED
    if _COMPILED is None:
        _COMPILED = _build()
    return _COMPILED


def make_in_maps(x, router_w, lora_down, lora_up):
    x = np.asarray(x, np.float32)
    rwt_np, wbf_np, w8_np = _weights_maps(
        np.asarray(router_w, np.float32),
        np.asarray(lora_down, np.float32),
        np.asarray(lora_up, np.float32),
    )
    in_maps = []
    for i in range(N_CORES):
        xs = x[i * NB : (i + 1) * NB]                         # [C, CT, H]
        # (p, c, k, t) <- xs[c, t, k*128+p]
        xtd = np.ascontiguousarray(
            xs.reshape(C, CT, NK, P).transpose(3, 0, 2, 1).reshape(P, C * NK * CT)
        ).astype(BF16_NP)
        cls = xs[:, 0, :]                                     # [NB, H]
        cls_t = np.ascontiguousarray(
            cls.reshape(NB, NK, P).transpose(2, 1, 0).reshape(P, NK * NB)
        ).astype(np.float32)
        wf32_np = np.ascontiguousarray(np.concatenate([cls_t, rwt_np], axis=1))
        in_maps.append(
            {"x_in": xtd, "wf32": wf32_np, "wbf": wbf_np, "w8": w8_np}
        )
    return in_maps


def unshard_one(y_np):
    """[P, C*NK*CT] h-major device output -> [NB, L, H] float32."""
    y = np.asarray(y_np, np.float32).reshape(P, C, NK, CT)
    return np.ascontiguousarray(y.transpose(1, 3, 2, 0)).reshape(NB, L, H)


def kernel(x, router_w, lora_down, lora_up):
    nc = get_compiled()
    in_maps = make_in_maps(x, router_w, lora_down, lora_up)
    res = run_bass_kernel_spmd(nc, in_maps, core_ids=list(range(N_CORES)))
    out = np.empty((B, L, H), np.float32)
    for i in range(N_CORES):
        out[i * NB : (i + 1) * NB] = unshard_one(res.results[i]["y_out"])
    return out
